# revision 1
# baseline (speedup 1.0000x reference)
"""Multi-head self-attention (B=2, T=2048, D=2048, H=16, RoPE, causal)
as a Bass/Tile kernel running SPMD on 8 trn2 NeuronCores.

Sharding: tensor-parallel over heads (2 heads per core). Each core
computes its heads' Q/K/V projections, RoPE, causal attention, and a
partial out-projection over its 256 feature columns; the host sums the
8 partial outputs (all-reduce equivalent).

Dataflow (per core, per batch):
  - projections in "T-layout" (feature dim on partitions, time on free):
    qT/kT = RoPE(W x^T); V transposed to natural layout via PE.
  - scores computed directly transposed: S^T[tk, tq] = K_j^T.T-free ...
    one 128-contraction matmul per (key-chunk, 512-wide q-group), so the
    exp output P^T is immediately the PV matmul's moving operand --
    no P transposes. Softmax skips max-subtraction (scores are O(10)).
  - row sums Z via a [128,1] ones matmul accumulated in PSUM across
    key chunks; reciprocal on DVE; partition-broadcast on GpSimd;
    normalization fused with the PSUM->SBUF move of the PV result.
  - out-projection accumulates the two head-chunks in PSUM, partial
    result DMA'd out; host sums partials across cores.
"""

import sys

sys.path.insert(0, "/opt/trn_rl_repo")

import ml_dtypes
import numpy as np

import concourse.bass as bass
import concourse.bass_isa as bass_isa
import concourse.mybir as mybir
import concourse.tile as tile
from concourse.bass_utils import run_bass_kernel_spmd
from concourse.tile_rust import add_dep_helper


def _absorb(eng, producers):
    """Emit engine-nops sync-depending on <=2 producers each so the engine
    observes those semaphores; later same-engine instructions then elide
    the waits (ISA instructions carry at most ~2 wait slots). Returns the
    nops; order consumers after them with _after()."""
    producers = [p for p in producers if p is not None]
    nops = []
    for i in range(0, len(producers), 2):
        nop = eng.nop()
        for p in producers[i : i + 2]:
            add_dep_helper(nop.ins, p.ins, sync=True, reason="wait-absorb")
        nops.append(nop)
    return nops


def _after(inst, nops):
    """Order `inst` after absorber nops (scheduler-only edges)."""
    for nop in nops:
        add_dep_helper(inst.ins, nop.ins, sync=False, reason="wait-absorb-order")


_SPILL_TYPES = (
    "InstDMACopy",
    "InstMatmult",
    "InstLdweights",
    "InstNoOp",
    "InstMemset",
    "InstPartitionBroadcast",
    "InstPartitionAllReduce",
    "InstTensorScalarAffineSelect",
)


def _legalize_waits(nc):
    """Walrus codegen rejects >2 sync waits on DMA/matmul/nop-class
    instructions, and Tile's pool-recycle waits bypass its own elision.
    Two fixes, both sound w.r.t. per-engine program order:
      1. strip waits already dominated by an earlier same-engine wait
      2. spill excess waits (>2) onto freshly inserted same-engine NoOps
         placed immediately before the offending instruction
    """
    # NOTE: wait-stripping by same-engine dominance is UNSOUND here --
    # Tile recycles semaphores mid-kernel (pool close -> sem_clear), so
    # values are not monotonic. Only the spill transformation is safe.
    do_strip = False
    do_spill = True
    seen = {}  # engine -> {sem_name: max_waited_value}
    spill_id = [0]
    for bb in nc.m.functions[0].blocks:
        new_insts = []
        for inst in bb.instructions:
            si = getattr(inst, "sync_info", None)
            if si is None or not si.on_wait:
                new_insts.append(inst)
                continue
            eng = getattr(inst, "engine", None)
            emap = seen.setdefault(str(eng), {})
            kept = []
            for w in si.on_wait:
                if (
                    w.sync_type == "semaphore"
                    and w.wait_mode == "sem-ge-imm"
                    and w.wait_reg is None
                    and w.wait_value is not None
                ):
                    if do_strip and emap.get(w.ant_name, -1) >= w.wait_value:
                        continue  # dominated: drop
                    emap[w.ant_name] = w.wait_value
                kept.append(w)
            if do_spill and len(kept) > 1 and eng is not None:
                excess, kept = kept[:-1], kept[-1:]
                for w in excess:
                    spill_id[0] += 1
                    nop = mybir.InstNoOp(
                        name=f"I-wspill-{spill_id[0]}",
                        ins=[],
                        outs=[],
                        engine=eng,
                    )
                    nop.sync_info = mybir.SyncInfo(on_wait=[w], on_update=[])
                    new_insts.append(nop)
            if len(kept) != len(si.on_wait):
                si.on_wait[:] = kept
            new_insts.append(inst)
        if len(new_insts) != len(bb.instructions):
            bb.instructions[:] = new_insts

B, T, D, H, HD = 2, 2048, 2048, 16, 128
NCORES = 8
HPC = H // NCORES            # heads per core = 2
M_PC = HPC * HD              # per-core feature slice = 256
BT = B * T                   # 4096
SCALE = HD ** -0.5
ROPE_THETA = 10000.0

F32 = mybir.dt.float32
BF16 = mybir.dt.bfloat16
BF16_NP = ml_dtypes.bfloat16

TB = 512                     # t-block for projections / q-groups
NTB_B = T // TB              # 4 t-blocks per batch
NMC = D // 128               # 16 contraction chunks
NKC = T // 128               # 16 key chunks per batch


def build_program():
    nc = bass.Bass()

    xT_d = nc.declare_dram_parameter("xT", [D, BT], BF16, isOutput=False)
    perm_d = nc.declare_dram_parameter("permM", [HD, HD], BF16, isOutput=False)
    negm_d = nc.declare_dram_parameter("negmM", [128, 128], F32, isOutput=False)
    fneg_d = nc.declare_dram_parameter("fnegM", [128, 128], F32, isOutput=False)
    id_d = nc.declare_dram_parameter("identM", [128, 128], BF16, isOutput=False)
    wq_d = nc.declare_dram_parameter("wqT", [D, M_PC], BF16, isOutput=False)
    wk_d = nc.declare_dram_parameter("wkT", [D, M_PC], BF16, isOutput=False)
    wv_d = nc.declare_dram_parameter("wvT", [D, M_PC], BF16, isOutput=False)
    wo_d = nc.declare_dram_parameter("woT", [M_PC, D], BF16, isOutput=False)
    cos_d = nc.declare_dram_parameter("cosT", [HD, T], F32, isOutput=False)
    sinh_d = nc.declare_dram_parameter("sinhT", [HD, T], F32, isOutput=False)
    out_d = nc.declare_dram_parameter("partialT", [D, BT], F32, isOutput=True)

    xT_v = xT_d.rearrange("(c p) t -> p c t", p=128)      # [128, 16, BT]
    wq_v = wq_d.rearrange("(c p) n -> p c n", p=128)      # [128, 16, 256]
    wk_v = wk_d.rearrange("(c p) n -> p c n", p=128)
    wv_v = wv_d.rearrange("(c p) n -> p c n", p=128)
    wo_v = wo_d.rearrange("(c p) n -> p c n", p=128)      # [128, 2, 2048]
    out_v = out_d.rearrange("(c p) t -> p c t", p=128)    # [128, 16, BT]

    with tile.TileContext(nc) as tc:
        with (
            tc.tile_pool(name="wpool", bufs=1) as wpool,
            tc.tile_pool(name="big", bufs=1) as big,
            tc.tile_pool(name="rp", bufs=2) as rp,
            tc.tile_pool(name="attn_sb", bufs=3) as asb,
            tc.tile_pool(name="z_sb", bufs=2) as zsb,
            tc.tile_pool(name="fs_sb", bufs=4) as fsb,
        ):
            # ---- constants / weights ----
            wq_sb = wpool.tile([128, NMC, M_PC], BF16, tag="wq")
            wk_sb = wpool.tile([128, NMC, M_PC], BF16, tag="wk")
            wv_sb = wpool.tile([128, NMC, M_PC], BF16, tag="wv")
            nc.sync.dma_start(out=wq_sb, in_=wq_v)
            nc.sync.dma_start(out=wk_sb, in_=wk_v)
            nc.sync.dma_start(out=wv_sb, in_=wv_v)
            cos_sb = wpool.tile([128, T], F32, tag="cos")
            sinh_sb = wpool.tile([128, T], F32, tag="sinh")
            nc.sync.dma_start(out=cos_sb, in_=cos_d[:, :])
            nc.sync.dma_start(out=sinh_sb, in_=sinh_d[:, :])
            perm_sb = wpool.tile([HD, HD], BF16, tag="perm")
            nc.sync.dma_start(out=perm_sb, in_=perm_d[:, :])

            # constant tiles (host-provided; gpsimd is avoided entirely --
            # its tail sem-clear ISA doesn't encode on this toolchain)
            negm = wpool.tile([128, 128], F32, tag="negm")
            nc.sync.dma_start(out=negm, in_=negm_d[:, :])
            fullneg = wpool.tile([128, 128], F32, tag="fullneg")
            nc.sync.dma_start(out=fullneg, in_=fneg_d[:, :])
            ident = wpool.tile([128, 128], BF16, tag="ident")
            nc.sync.dma_start(out=ident, in_=id_d[:, :])
            ones_col = wpool.tile([128, 1], BF16, tag="ones_c")
            nc.vector.memset(ones_col, 1.0)
            ones_row = wpool.tile([1, 128], F32, tag="ones_r")
            nc.vector.memset(ones_row, 1.0)

            wo_sb = wpool.tile([128, HPC, D], BF16, tag="wo")
            wo_dma = nc.sync.dma_start(out=wo_sb, in_=wo_v)

            prev_x_dmas = []
            for b in range(B):
                t0 = b * T  # global t offset of this batch

                # persistent per-batch tensors (slots reused across b)
                qT = big.tile([128, HPC, T], BF16, tag="qT")   # [hd, h, t]
                kT = big.tile([128, HPC, T], BF16, tag="kT")
                vN = big.tile([128, HPC, NKC, HD], BF16, tag="vN")  # [tk, h, j, d]
                oT = big.tile([128, HPC, T], BF16, tag="oT")   # attn out, T-layout

                # ---------------- projections + RoPE ----------------
                # whole-batch x resident in SBUF, loaded as 16 disjoint
                # sub-DMAs (keeps per-DMA sync waits low). Before the slot
                # is recycled for batch 1, let SP observe batch 0's DMA
                # queue semaphores so the WAW waits collapse.
                xnops = _absorb(nc.sync, prev_x_dmas)
                xb = big.tile([128, NMC, T], BF16, tag="xb")
                prev_x_dmas = []
                for mc in range(NMC):
                    d = nc.sync.dma_start(
                        out=xb[:, mc, :], in_=xT_v[:, mc, t0 : t0 + T]
                    )
                    _after(d, xnops)
                    prev_x_dmas.append(d)
                with tc.tile_pool(name="proj_ps", bufs=1, space="PSUM") as pps, \
                     tc.tile_pool(name="vt_ps", bufs=1, space="PSUM") as vtp:
                    for tb in range(NTB_B):
                        ts_l = slice(tb * TB, (tb + 1) * TB)          # in-batch
                        ps = {}
                        for h in range(HPC):
                            ps["q", h] = pps.tile(
                                [128, TB], F32, tag=f"q{h}", name=f"ps_q{h}"
                            )
                            ps["k", h] = pps.tile(
                                [128, TB], F32, tag=f"k{h}", name=f"ps_k{h}"
                            )
                            ps["v", h] = pps.tile(
                                [128, TB], F32, tag=f"v{h}", name=f"ps_v{h}"
                            )
                        for mc in range(NMC):
                            for h in range(HPC):
                                hs = slice(h * HD, (h + 1) * HD)
                                for nm, wsb in (
                                    ("q", wq_sb),
                                    ("k", wk_sb),
                                    ("v", wv_sb),
                                ):
                                    nc.tensor.matmul(
                                        ps[nm, h],
                                        lhsT=wsb[:, mc, hs],
                                        rhs=xb[:, mc, ts_l],
                                        start=(mc == 0),
                                        stop=(mc == NMC - 1),
                                    )
                        for h in range(HPC):
                            # RoPE for q, k: half-swap via PE permutation
                            # matmul; fp32 combine on DVE; bf16 result
                            for nm, dest in (("q", qT), ("k", kT)):
                                raw = rp.tile([128, TB], BF16, tag="raw")
                                nc.scalar.activation(
                                    raw, ps[nm, h], mybir.ActivationFunctionType.Copy
                                )
                                swps = vtp.tile([128, TB], F32, tag="swps")
                                nc.tensor.matmul(
                                    swps, lhsT=perm_sb, rhs=raw,
                                    start=True, stop=True,
                                )
                                t1 = rp.tile([128, TB], F32, tag="t1")
                                nc.vector.tensor_mul(t1, raw, cos_sb[:, ts_l])
                                t2 = rp.tile([128, TB], F32, tag="t2")
                                nc.vector.tensor_mul(t2, swps, sinh_sb[:, ts_l])
                                nc.vector.tensor_add(dest[:, h, ts_l], t1, t2)
                            # V: cast to bf16 SBUF then PE-transpose to natural
                            vt_sb = rp.tile([128, TB], BF16, tag="vtmp")
                            nc.scalar.activation(
                                vt_sb, ps["v", h], mybir.ActivationFunctionType.Copy
                            )
                            for s in range(TB // 128):
                                j = tb * (TB // 128) + s
                                pst = vtp.tile([128, 128], BF16, tag="vt")
                                nc.tensor.transpose(
                                    pst, vt_sb[:, s * 128 : (s + 1) * 128], ident
                                )
                                nc.vector.tensor_copy(vN[:, h, j, :], pst)

                # ---------------- attention ----------------
                last_exp = last_omul = None
                with tc.tile_pool(name="st_ps", bufs=3, space="PSUM") as stp, \
                     tc.tile_pool(name="pv_ps", bufs=2, space="PSUM") as pvp, \
                     tc.tile_pool(name="z_ps", bufs=1, space="PSUM") as zpp:
                    for h in range(HPC):
                        for qg in range(NTB_B):
                            qs = slice(qg * TB, (qg + 1) * TB)  # in-batch q range
                            jmax = (TB // 128) * (qg + 1)
                            po = pvp.tile([128, TB], F32, tag="po")
                            zrow = zpp.tile([1, TB], F32, tag="zrow")
                            for j in range(jmax):
                                ks_ = slice(j * 128, (j + 1) * 128)
                                st = stp.tile([128, TB], F32, tag="st")
                                nc.tensor.matmul(
                                    st,
                                    lhsT=kT[:, h, ks_],
                                    rhs=qT[:, h, qs],
                                    start=True,
                                    stop=True,
                                )
                                # causal mask: diag triangle / full block
                                for i in range(TB // 128):
                                    qb = qg * (TB // 128) + i
                                    if j == qb or j > qb:
                                        blk = slice(i * 128, (i + 1) * 128)
                                        nc.vector.tensor_add(
                                            st[:, blk],
                                            st[:, blk],
                                            negm if j == qb else fullneg,
                                        )
                                pt = asb.tile([128, TB], BF16, tag="pt")
                                last_exp = nc.scalar.activation(
                                    pt,
                                    st,
                                    mybir.ActivationFunctionType.Exp,
                                    scale=SCALE,
                                )
                                nc.tensor.matmul(
                                    zrow,
                                    lhsT=ones_col,
                                    rhs=pt,
                                    start=(j == 0),
                                    stop=(j == jmax - 1),
                                )
                                nc.tensor.matmul(
                                    po,
                                    lhsT=vN[:, h, j, :],
                                    rhs=pt,
                                    start=(j == 0),
                                    stop=(j == jmax - 1),
                                )
                            # normalization: recip + PE ones-matmul broadcast
                            zrs = zsb.tile([1, TB], F32, tag="zrs")
                            nc.vector.reciprocal(zrs, zrow)
                            zbp = zpp.tile([128, TB], F32, tag="zbp")
                            nc.tensor.matmul(
                                zbp, lhsT=ones_row, rhs=zrs,
                                start=True, stop=True,
                            )
                            zbr = zsb.tile([128, TB], F32, tag="zbr")
                            nc.scalar.activation(
                                zbr, zbp, mybir.ActivationFunctionType.Copy
                            )
                            last_omul = nc.vector.tensor_mul(
                                oT[:, h, qs], po, zbr
                            )

                # ---------------- out-projection (partial) ----------------
                # let PE observe the attention-phase tail so the first
                # out-proj matmuls don't aggregate >2 waits
                onops = _absorb(nc.tensor, [last_exp, last_omul, wo_dma])
                with tc.tile_pool(name="fo_ps", bufs=1, space="PSUM") as fop:
                    for nb in range(D // 128):
                        nbs = slice(nb * 128, (nb + 1) * 128)
                        fo = {}
                        for m in range(HPC):
                            for tb in range(NTB_B):
                                tbs = slice(tb * TB, (tb + 1) * TB)
                                if m == 0:
                                    fo[tb] = fop.tile(
                                        [128, TB], F32, tag=f"fo{tb % 4}",
                                        name=f"fo{tb}",
                                    )
                                mm = nc.tensor.matmul(
                                    fo[tb],
                                    lhsT=wo_sb[:, m, nbs],
                                    rhs=oT[:, m, tbs],
                                    start=(m == 0),
                                    stop=(m == HPC - 1),
                                )
                                if nb == 0 and m == 0:
                                    _after(mm, onops)
                        for tb in range(NTB_B):
                            fs = fsb.tile([128, TB], F32, tag=f"fs{tb}")
                            if tb % 2 == 0:
                                nc.vector.tensor_copy(fs, fo[tb])
                            else:
                                nc.scalar.activation(
                                    fs, fo[tb], mybir.ActivationFunctionType.Copy
                                )
                            nc.sync.dma_start(
                                out=out_v[:, nb, t0 + tb * TB : t0 + (tb + 1) * TB],
                                in_=fs,
                            )
    _legalize_waits(nc)
    return nc


_NC_CACHE = None


def _get_program():
    global _NC_CACHE
    if _NC_CACHE is None:
        _NC_CACHE = build_program()
    return _NC_CACHE


def _rope_tables():
    inv_freq = 1.0 / (ROPE_THETA ** (np.arange(0, HD, 2, dtype=np.float32) / HD))
    freqs = np.arange(T, dtype=np.float32)[:, None] * inv_freq[None, :]  # (T, 64)
    emb = np.concatenate([freqs, freqs], axis=-1)                        # (T, 128)
    cosT = np.ascontiguousarray(np.cos(emb).T.astype(np.float32))        # [128, T]
    sinT = np.sin(emb).T.astype(np.float32)
    sinhT = np.ascontiguousarray(
        np.concatenate([-sinT[: HD // 2], sinT[HD // 2 :]], axis=0)
    )
    return cosT, sinhT


def kernel(x, Wq, Wk, Wv, Wo, **run_kwargs):
    x = np.asarray(x, dtype=np.float32)
    Wq = np.asarray(Wq, dtype=np.float32)
    Wk = np.asarray(Wk, dtype=np.float32)
    Wv = np.asarray(Wv, dtype=np.float32)
    Wo = np.asarray(Wo, dtype=np.float32)

    nc = _get_program()
    cosT, sinhT = _rope_tables()
    xT = np.ascontiguousarray(x.reshape(BT, D).T).astype(BF16_NP)  # [D, BT]
    permM = np.zeros((HD, HD), dtype=BF16_NP)
    for m in range(HD):
        permM[(m + HD // 2) % HD, m] = 1.0  # out[m] = in[(m+64)%128]
    # S^T[tk, tq] causal masks: keep where tq(col) >= tk(row)
    r = np.arange(128)
    negmM = np.where(r[None, :] >= r[:, None], 0.0, -1e30).astype(np.float32)
    fnegM = np.full((128, 128), -1e30, dtype=np.float32)
    identM = np.eye(128, dtype=BF16_NP)

    in_maps = []
    for c in range(NCORES):
        sl = slice(c * M_PC, (c + 1) * M_PC)
        in_maps.append(
            {
                "xT": xT,
                "permM": permM,
                "negmM": negmM,
                "fnegM": fnegM,
                "identM": identM,
                "wqT": np.ascontiguousarray(Wq[sl, :].T).astype(BF16_NP),
                "wkT": np.ascontiguousarray(Wk[sl, :].T).astype(BF16_NP),
                "wvT": np.ascontiguousarray(Wv[sl, :].T).astype(BF16_NP),
                "woT": np.ascontiguousarray(Wo[:, sl].T).astype(BF16_NP),
                "cosT": cosT,
                "sinhT": sinhT,
            }
        )

    res = run_bass_kernel_spmd(nc, in_maps, list(range(NCORES)), **run_kwargs)
    acc = np.zeros((D, BT), dtype=np.float32)
    for c in range(NCORES):
        acc += res.results[c]["partialT"]
    out = np.ascontiguousarray(acc.T).reshape(B, T, D)
    if run_kwargs:
        return out, res
    return out



# revision 18
# speedup vs baseline: 1.3384x; 1.3384x over previous
"""Multi-head self-attention (B=2, T=2048, D=2048, H=16, RoPE, causal)
as a Bass/Tile kernel running SPMD on 8 trn2 NeuronCores.

Sharding: tensor-parallel over heads (2 heads per core). Each core
computes its heads' Q/K/V projections, RoPE, causal attention, and a
partial out-projection over its 256 feature columns; the host sums the
8 partial outputs (all-reduce equivalent).

Dataflow (per core, per batch):
  - x streamed per 512-wide t-block ([128, 16, 512] SBUF tiles, 4 tags);
    the first block's DMA is interleaved per-contraction-chunk with the
    weight loads so the PE starts ~2us in.
  - Q/K projections in "T-layout" (feature dim on partitions, time on
    free); RoPE rotate-half via a PE permutation matmul, combines on DVE
    in bf16 (2x mode where operands allow).
  - V projected directly in natural layout ([tk, d]): lhsT = x chunk,
    rhs = Wv slice -- no PE transposes.
  - scores computed transposed: S^T[tk, tq] per (key-chunk, q-group).
    Chunks are narrowed to the causal region (exact 136-block lower
    triangle, no fully-masked work); only the diagonal 128x128 block
    gets a mask add. The two heads' chunk streams are interleaved so
    the PE always has ~1.3us of work while exp round-trips through
    DVE/Act. Z row sums via a [128,1] ones matmul accumulated in PSUM.
  - normalization is a post-pass: po -> oT (unnormalized, DVE cast),
    1/Z table on DVE, then per (h, tb) a ones-row broadcast matmul
    (borrowing score-tile PSUM slots) + in-place DVE multiply.
  - out-projection accumulates the two head-chunks in PSUM; partial
    result cast to f16 and DMA'd out; host sums partials across cores.
"""

import sys

sys.path.insert(0, "/opt/trn_rl_repo")

import ml_dtypes
import numpy as np

import concourse.bass as bass
import concourse.bass_isa as bass_isa
import concourse.mybir as mybir
import concourse.tile as tile
from concourse.bass_utils import run_bass_kernel_spmd
from concourse.tile_rust import add_dep_helper

_SPILL_TYPES = (
    "InstDMACopy",
    "InstMatmult",
    "InstLdweights",
    "InstNoOp",
    "InstMemset",
    "InstPartitionBroadcast",
    "InstPartitionAllReduce",
    "InstTensorScalarAffineSelect",
)


def _legalize_waits(nc):
    """Walrus codegen rejects >2 sync waits on DMA/matmul/nop-class
    instructions, and Tile's pool-recycle waits bypass its own elision.
    Spill excess waits (>1) onto freshly inserted same-engine NoOps
    placed immediately before the offending instruction (sound w.r.t.
    per-engine program order)."""
    spill_id = [0]
    for bb in nc.m.functions[0].blocks:
        new_insts = []
        for inst in bb.instructions:
            si = getattr(inst, "sync_info", None)
            if si is None or not si.on_wait:
                new_insts.append(inst)
                continue
            eng = getattr(inst, "engine", None)
            kept = list(si.on_wait)
            if len(kept) > 1 and eng is not None:
                excess, kept = kept[:-1], kept[-1:]
                for w in excess:
                    spill_id[0] += 1
                    nop = mybir.InstNoOp(
                        name=f"I-wspill-{spill_id[0]}",
                        ins=[],
                        outs=[],
                        engine=eng,
                    )
                    nop.sync_info = mybir.SyncInfo(on_wait=[w], on_update=[])
                    new_insts.append(nop)
            if len(kept) != len(si.on_wait):
                si.on_wait[:] = kept
            new_insts.append(inst)
        if len(new_insts) != len(bb.instructions):
            bb.instructions[:] = new_insts


_PHASE_MARKS = []  # (phase_label, last_inst_index_before_phase) - profiling aid


def _mark(nc, label):
    n = -1
    for fn in nc.m.functions:
        for bb in fn.blocks:
            for ins in bb.instructions:
                if ins.name.startswith("I-"):
                    try:
                        n = max(n, int(ins.name[2:]))
                    except ValueError:
                        pass
    _PHASE_MARKS.append((label, n))


B, T, D, H, HD = 2, 2048, 2048, 16, 128
NCORES = 8
HPC = H // NCORES            # heads per core = 2
M_PC = HPC * HD              # per-core feature slice = 256
BT = B * T                   # 4096
SCALE = HD ** -0.5
ROPE_THETA = 10000.0

F32 = mybir.dt.float32
F16 = mybir.dt.float16
BF16 = mybir.dt.bfloat16
BF16_NP = ml_dtypes.bfloat16

TB = 512                     # t-block for projections / q-groups
NTB_B = T // TB              # 4 t-blocks per batch
NMC = D // 128               # 16 contraction chunks
NKC = T // 128               # 16 key chunks per batch
JPG = TB // 128              # key chunks per q-group width = 4

Copy = mybir.ActivationFunctionType.Copy
Exp = mybir.ActivationFunctionType.Exp


def build_program():
    nc = bass.Bass()

    xT_d = nc.declare_dram_parameter("xT", [D, BT], BF16, isOutput=False)
    perm_d = nc.declare_dram_parameter("permM", [HD, HD], BF16, isOutput=False)
    negm_d = nc.declare_dram_parameter("negmM", [128, 128], F32, isOutput=False)
    # wq and wk concatenated so one DMA covers both (halves SP-seq time
    # on the critical startup path)
    wqk_d = nc.declare_dram_parameter(
        "wqkT", [D, 2 * M_PC], BF16, isOutput=False
    )
    wv_d = nc.declare_dram_parameter("wvT", [D, M_PC], BF16, isOutput=False)
    wo_d = nc.declare_dram_parameter("woT", [M_PC, D], BF16, isOutput=False)
    cos_d = nc.declare_dram_parameter("cosT", [HD, T], BF16, isOutput=False)
    sinh_d = nc.declare_dram_parameter("sinhT", [HD, T], BF16, isOutput=False)
    out_d = nc.declare_dram_parameter("partialT", [D, BT], F16, isOutput=True)

    xT_v = xT_d.rearrange("(c p) t -> p c t", p=128)      # [128, 16, BT]
    wqk_v = wqk_d.rearrange("(c p) n -> p c n", p=128)    # [128, 16, 512]
    wv_v = wv_d.rearrange("(c p) n -> p c n", p=128)
    wo_v = wo_d.rearrange("(c p) n -> p c n", p=128)      # [128, 2, 2048]
    out_v = out_d.rearrange("(c p) t -> p c t", p=128)    # [128, 16, BT]

    with tile.TileContext(nc) as tc:
        with (
            tc.tile_pool(name="wpool", bufs=1) as wpool,
            tc.tile_pool(name="xp", bufs=1) as xp,
            tc.tile_pool(name="big", bufs=1) as big,
            tc.tile_pool(name="rp", bufs=2) as rp,
            tc.tile_pool(name="attn_sb", bufs=4) as asb,
            tc.tile_pool(name="fs_sb", bufs=3) as fsb,
        ):
            # ---- weights + first x block, interleaved in graduated mc
            # groups (fast pipeline fill, then few big SP-cheap DMAs) ----
            wqk_sb = wpool.tile([128, NMC, 2 * M_PC], BF16, tag="wqk")
            wv_sb = wpool.tile([128, NMC, M_PC], BF16, tag="wv")
            x_tiles = {}
            xt0 = xp.tile([128, NMC, TB], BF16, tag="x0", name="x_b0_t0")
            x_tiles[(0, 0)] = xt0
            for lo, hi in ((0, 1), (1, 2), (2, 4), (4, 8), (8, 12), (12, 16)):
                nc.sync.dma_start(
                    out=wqk_sb[:, lo:hi, :], in_=wqk_v[:, lo:hi, :]
                )
                nc.sync.dma_start(
                    out=xt0[:, lo:hi, :], in_=xT_v[:, lo:hi, 0:TB]
                )

            cos_sb = wpool.tile([128, T], BF16, tag="cos")
            sinh_sb = wpool.tile([128, T], BF16, tag="sinh")
            perm_sb = wpool.tile([HD, HD], BF16, tag="perm")
            nc.sync.dma_start(out=perm_sb, in_=perm_d[:, :])
            nc.sync.dma_start(out=cos_sb[:, 0:TB], in_=cos_d[:, 0:TB])
            nc.sync.dma_start(out=sinh_sb[:, 0:TB], in_=sinh_d[:, 0:TB])

            def load_x(b, tb):
                t = xp.tile(
                    [128, NMC, TB], BF16, tag=f"x{tb}", name=f"x_b{b}_t{tb}"
                )
                x_tiles[(b, tb)] = t
                lo = b * T + tb * TB
                for m0 in range(0, NMC, 4):
                    nc.sync.dma_start(
                        out=t[:, m0 : m0 + 4, :],
                        in_=xT_v[:, m0 : m0 + 4, lo : lo + TB],
                    )

            # wv rides alongside tb0's V matmuls; x block 1 follows
            for m0 in range(0, NMC, 4):
                nc.sync.dma_start(
                    out=wv_sb[:, m0 : m0 + 4, :], in_=wv_v[:, m0 : m0 + 4, :]
                )
            load_x(0, 1)
            nc.sync.dma_start(out=cos_sb[:, TB:], in_=cos_d[:, TB:])
            nc.sync.dma_start(out=sinh_sb[:, TB:], in_=sinh_d[:, TB:])
            negm = wpool.tile([128, 128], F32, tag="negm")
            nc.sync.dma_start(out=negm, in_=negm_d[:, :])
            ones_col = wpool.tile([128, 1], BF16, tag="ones_c")
            nc.vector.memset(ones_col, 1.0)
            ones_row = wpool.tile([1, 128], BF16, tag="ones_r")
            nc.vector.memset(ones_row, 1.0)
            # 1/Z table: [1, HPC*T], column h*T + t (kept on partition 0)
            zrs_tab = wpool.tile([1, HPC * T], BF16, tag="zrs")

            for tb in range(2, NTB_B):
                load_x(0, tb)

            wo_sb = wpool.tile([128, HPC, D], BF16, tag="wo")
            nc.sync.dma_start(out=wo_sb, in_=wo_v)

            for b in range(B):
                t0 = b * T  # global t offset of this batch
                _mark(nc, f"b{b}_proj")

                # persistent per-batch tensors (slots reused across b)
                qT = big.tile([128, HPC, T], BF16, tag="qT")   # [hd, h, t]
                kT = big.tile([128, HPC, T], BF16, tag="kT")
                vN = big.tile([128, NKC, M_PC], BF16, tag="vN")  # [tk, j, n]
                oT = big.tile([128, HPC, T], BF16, tag="oT")   # attn out

                # ---------------- projections + RoPE ----------------
                with tc.tile_pool(name="qk_ps", bufs=1, space="PSUM") as qkp, \
                     tc.tile_pool(name="v_ps", bufs=1, space="PSUM") as vps, \
                     tc.tile_pool(name="sw_ps", bufs=2, space="PSUM") as swp:
                    for tb in range(NTB_B):
                        xt = x_tiles[(b, tb)]
                        ts_l = slice(tb * TB, (tb + 1) * TB)   # in-batch
                        ps = {}
                        for h in range(HPC):
                            for nm in ("q", "k"):
                                ps[nm, h] = qkp.tile(
                                    [128, TB], F32, tag=f"{nm}{h}",
                                    name=f"ps_{nm}{h}",
                                )
                        for mc in range(NMC):
                            for h in range(HPC):
                                for ni, nm in ((0, "q"), (1, "k")):
                                    hs = slice(
                                        ni * M_PC + h * HD,
                                        ni * M_PC + (h + 1) * HD,
                                    )
                                    nc.tensor.matmul(
                                        ps[nm, h],
                                        lhsT=wqk_sb[:, mc, hs],
                                        rhs=xt[:, mc, :],
                                        start=(mc == 0),
                                        stop=(mc == NMC - 1),
                                    )
                        # raw bf16 casts early on Act (deps ready now)
                        raws = {}
                        for nm in ("q", "k"):
                            for h in range(HPC):
                                raw = rp.tile(
                                    [128, TB], BF16, tag=f"raw{nm}{h}"
                                )
                                nc.scalar.activation(raw, ps[nm, h], Copy)
                                raws[nm, h] = raw
                        # RoPE before V: the qk/sw PSUM banks drain while the
                        # PE streams V matmuls, so the next phase's bank-WAR
                        # waits resolve before the PE gets there
                        for nm, dest in (("q", qT), ("k", kT)):
                            for h in range(HPC):
                                raw = raws[nm, h]
                                swps = swp.tile([128, TB], F32, tag="sw")
                                nc.tensor.matmul(
                                    swps, lhsT=perm_sb, rhs=raw,
                                    start=True, stop=True,
                                )
                                t2 = rp.tile([128, TB], BF16, tag="t2")
                                nc.vector.tensor_mul(t2, swps, sinh_sb[:, ts_l])
                                t1 = rp.tile([128, TB], BF16, tag="t1")
                                nc.vector.tensor_mul(t1, raw, cos_sb[:, ts_l])
                                nc.vector.tensor_add(dest[:, h, ts_l], t1, t2)
                        # V directly in natural layout: per 128-wide tk chunk
                        for s in range(JPG):
                            j = tb * JPG + s
                            pv = vps.tile(
                                [128, M_PC], F32, tag=f"v{s % 2}",
                                name=f"ps_v{s % 2}",
                            )
                            for mc in range(NMC):
                                nc.tensor.matmul(
                                    pv,
                                    lhsT=xt[:, mc, s * 128 : (s + 1) * 128],
                                    rhs=wv_sb[:, mc, :],
                                    start=(mc == 0),
                                    stop=(mc == NMC - 1),
                                )
                            nc.scalar.activation(vN[:, j, :], pv, Copy)

                if b == 0:
                    # b1's x loads: emitted here so they ride the SP queue
                    # ahead of b0's output stores
                    for tb in range(NTB_B):
                        load_x(1, tb)

                _mark(nc, f"b{b}_attn")
                # ---------------- attention ----------------
                # heads interleaved per key-chunk; Z/PV trail the score/exp
                # stream by one chunk, software-pipelined ACROSS q-groups so
                # the PE never drains at a group boundary.
                with tc.tile_pool(name="st_ps", bufs=3, space="PSUM") as stp, \
                     tc.tile_pool(name="pv_ps", bufs=1, space="PSUM") as pvp, \
                     tc.tile_pool(name="z_ps", bufs=1, space="PSUM") as zpp, \
                     tc.tile_pool(name="zb_ps", bufs=1, space="PSUM") as zbq:
                    # (qg, j) chunk schedule, flattened
                    sched = [
                        (qg, j)
                        for qg in range(NTB_B)
                        for j in range(JPG * (qg + 1))
                    ]
                    po = {}
                    zrow = {}
                    prev = None  # (qg, j, {h: (pt, off)})
                    norm_q = []  # pending (qg, h) normalization steps

                    def norm_step():
                        """One 1/Z broadcast + in-place oT scale; spaced one
                        chunk-iteration apart so the single zbp bank's WAR
                        (previous DVE mul) is always resolved."""
                        nqg, h = norm_q.pop(0)
                        qs0 = nqg * TB
                        zbp = zbq.tile([128, TB], F32, tag="zbp", name="zbp")
                        nc.tensor.matmul(
                            zbp,
                            lhsT=ones_row,
                            rhs=zrs_tab[0:1, h * T + qs0 : h * T + qs0 + TB],
                            start=True,
                            stop=True,
                        )
                        nc.vector.tensor_mul(
                            oT[:, h, qs0 : qs0 + TB],
                            oT[:, h, qs0 : qs0 + TB],
                            zbp,
                        )

                    def zpv_step(pqg, pj, pts):
                        """Z + PV matmuls for the trailing chunk; on the last
                        chunk of a group, also emit recip + po drain."""
                        pjmax = JPG * (pqg + 1)
                        for h in range(HPC):
                            ptp, offp = pts[h]
                            nc.tensor.matmul(
                                zrow[pqg, h][:, offp:],
                                lhsT=ones_col,
                                rhs=ptp[:, offp:],
                                start=(pj == 0),
                                stop=(pj == pjmax - 1),
                            )
                            nc.tensor.matmul(
                                po[pqg, h][:, offp:],
                                lhsT=vN[:, pj, h * HD : (h + 1) * HD],
                                rhs=ptp[:, offp:],
                                start=(pj == 0),
                                stop=(pj == pjmax - 1),
                            )
                        if pj == pjmax - 1:
                            qs0 = pqg * TB
                            for h in range(HPC):
                                with nc.allow_low_precision(
                                    reason="bf16 1/Z: 0.4% rel, in tolerance"
                                ):
                                    nc.vector.reciprocal(
                                        zrs_tab[
                                            0:1, h * T + qs0 : h * T + qs0 + TB
                                        ],
                                        zrow[pqg, h],
                                    )
                                nc.vector.tensor_copy(
                                    oT[:, h, qs0 : qs0 + TB], po[pqg, h]
                                )
                                norm_q.append((pqg, h))

                    for qg, j in sched:
                        if j == 0:
                            for h in range(HPC):
                                po[qg, h] = pvp.tile(
                                    [128, TB], F32, tag=f"po{h}", name=f"po{h}"
                                )
                                zrow[qg, h] = zpp.tile(
                                    [1, TB], F32, tag=f"z{h}", name=f"z{h}"
                                )
                        qs0 = qg * TB
                        cur = {}
                        for h in range(HPC):
                            off = max(0, (j - JPG * qg) * 128)
                            ks = slice(j * 128, (j + 1) * 128)
                            st = stp.tile([128, TB], F32, tag="st")
                            nc.tensor.matmul(
                                st[:, off:],
                                lhsT=kT[:, h, ks],
                                rhs=qT[:, h, qs0 + off : qs0 + TB],
                                start=True,
                                stop=True,
                            )
                            if j >= JPG * qg:  # diagonal 128-block mask
                                nc.vector.tensor_add(
                                    st[:, off : off + 128],
                                    st[:, off : off + 128],
                                    negm,
                                )
                            pt = asb.tile([128, TB], BF16, tag="pt")
                            nc.scalar.activation(
                                pt[:, off:], st[:, off:], Exp, scale=SCALE
                            )
                            cur[h] = (pt, off)
                        if prev is not None:
                            zpv_step(*prev)
                        if norm_q:
                            norm_step()
                        prev = (qg, j, cur)
                    zpv_step(*prev)
                    while norm_q:
                        norm_step()

                _mark(nc, f"b{b}_outproj")
                # ---------------- out-projection (partial) ----------------
                # output stores grouped 4 column-blocks per DMA (SP-seq time
                # per DMA is the store-path bottleneck, not bandwidth)
                with tc.tile_pool(name="fo_ps", bufs=1, space="PSUM") as fop:
                    for tb in range(NTB_B):
                        tbs = slice(tb * TB, (tb + 1) * TB)
                        fs = None
                        for nb in range(D // 128):
                            nbs = slice(nb * 128, (nb + 1) * 128)
                            fo = fop.tile(
                                [128, TB], F32, tag=f"fo{nb % 3}",
                                name=f"fo{nb % 3}",
                            )
                            for m in range(HPC):
                                nc.tensor.matmul(
                                    fo,
                                    lhsT=wo_sb[:, m, nbs],
                                    rhs=oT[:, m, tbs],
                                    start=(m == 0),
                                    stop=(m == HPC - 1),
                                )
                            if nb % 4 == 0:
                                fs = fsb.tile(
                                    [128, 4, TB], F16, tag="fs"
                                )
                            if nb % 2 == 0:
                                nc.vector.tensor_copy(fs[:, nb % 4, :], fo)
                            else:
                                nc.scalar.activation(fs[:, nb % 4, :], fo, Copy)
                            if nb % 4 == 3:
                                nc.sync.dma_start(
                                    out=out_v[
                                        :,
                                        nb - 3 : nb + 1,
                                        t0 + tb * TB : t0 + (tb + 1) * TB,
                                    ],
                                    in_=fs,
                                )
    _legalize_waits(nc)
    return nc


_NC_CACHE = None


def _get_program():
    global _NC_CACHE
    if _NC_CACHE is None:
        _NC_CACHE = build_program()
    return _NC_CACHE


def _rope_tables():
    inv_freq = 1.0 / (ROPE_THETA ** (np.arange(0, HD, 2, dtype=np.float32) / HD))
    freqs = np.arange(T, dtype=np.float32)[:, None] * inv_freq[None, :]  # (T, 64)
    emb = np.concatenate([freqs, freqs], axis=-1)                        # (T, 128)
    cosT = np.ascontiguousarray(np.cos(emb).T).astype(BF16_NP)           # [128, T]
    sinT = np.sin(emb).T.astype(np.float32)
    sinhT = np.ascontiguousarray(
        np.concatenate([-sinT[: HD // 2], sinT[HD // 2 :]], axis=0)
    ).astype(BF16_NP)
    return cosT, sinhT


def kernel(x, Wq, Wk, Wv, Wo, **run_kwargs):
    x = np.asarray(x, dtype=np.float32)
    Wq = np.asarray(Wq, dtype=np.float32)
    Wk = np.asarray(Wk, dtype=np.float32)
    Wv = np.asarray(Wv, dtype=np.float32)
    Wo = np.asarray(Wo, dtype=np.float32)

    nc = _get_program()
    cosT, sinhT = _rope_tables()
    xT = np.ascontiguousarray(x.reshape(BT, D).T).astype(BF16_NP)  # [D, BT]
    permM = np.zeros((HD, HD), dtype=BF16_NP)
    for m in range(HD):
        permM[(m + HD // 2) % HD, m] = 1.0  # out[m] = in[(m+64)%128]
    # S^T[tk, tq] causal mask for the diagonal block: keep tq(col) >= tk(row)
    r = np.arange(128)
    negmM = np.where(r[None, :] >= r[:, None], 0.0, -1e30).astype(np.float32)

    in_maps = []
    for c in range(NCORES):
        sl = slice(c * M_PC, (c + 1) * M_PC)
        in_maps.append(
            {
                "xT": xT,
                "permM": permM,
                "negmM": negmM,
                "wqkT": np.ascontiguousarray(
                    np.concatenate([Wq[sl, :].T, Wk[sl, :].T], axis=1)
                ).astype(BF16_NP),
                "wvT": np.ascontiguousarray(Wv[sl, :].T).astype(BF16_NP),
                "woT": np.ascontiguousarray(Wo[:, sl].T).astype(BF16_NP),
                "cosT": cosT,
                "sinhT": sinhT,
            }
        )

    res = run_bass_kernel_spmd(nc, in_maps, list(range(NCORES)), **run_kwargs)
    acc = np.zeros((D, BT), dtype=np.float32)
    for c in range(NCORES):
        acc += res.results[c]["partialT"].astype(np.float32)
    out = np.ascontiguousarray(acc.T).reshape(B, T, D)
    if run_kwargs:
        return out, res
    return out


# revision 34
# speedup vs baseline: 1.3755x; 1.0278x over previous
"""Multi-head self-attention (B=2, T=2048, D=2048, H=16, RoPE, causal)
as a Bass/Tile kernel running SPMD on 8 trn2 NeuronCores.

Sharding: tensor-parallel over heads (2 heads per core). Each core
computes its heads' Q/K/V projections, RoPE, causal attention, and a
partial out-projection over its 256 feature columns; the host sums the
8 partial outputs (all-reduce equivalent).

Dataflow (per core, per batch):
  - x streamed per 512-wide t-block ([128, 16, 512] SBUF tiles, 4 tags);
    the first block's DMA is interleaved per-contraction-chunk with the
    weight loads so the PE starts ~2us in.
  - Q/K projections in "T-layout" (feature dim on partitions, time on
    free); RoPE rotate-half via a PE permutation matmul, combines on DVE
    in bf16 (2x mode where operands allow).
  - V projected directly in natural layout ([tk, d]): lhsT = x chunk,
    rhs = Wv slice -- no PE transposes.
  - scores computed transposed: S^T[tk, tq] per (key-chunk, q-group).
    Chunks are narrowed to the causal region (exact 136-block lower
    triangle, no fully-masked work); only the diagonal 128x128 block
    gets a mask add. The two heads' chunk streams are interleaved so
    the PE always has ~1.3us of work while exp round-trips through
    DVE/Act. Z row sums via a [128,1] ones matmul accumulated in PSUM.
  - normalization trails each q-group: po -> oT (unnormalized cast),
    1/Z table via DVE reciprocal, then a ones-row broadcast matmul
    (riding the po PSUM slots between groups) + in-place DVE multiply.
  - out-projection accumulates the two head-chunks in PSUM; partial
    result cast to f16 and DMA'd out; host sums partials across cores.
"""

import sys

sys.path.insert(0, "/opt/trn_rl_repo")

import ml_dtypes
import numpy as np

import concourse.bass as bass
import concourse.mybir as mybir
import concourse.tile as tile
from concourse.bass_utils import run_bass_kernel_spmd


def _legalize_waits(nc):
    """Walrus codegen rejects >2 sync waits on DMA/matmul/nop-class
    instructions, and Tile's pool-recycle waits bypass its own elision.
    Spill excess waits (>1) onto freshly inserted same-engine NoOps
    placed immediately before the offending instruction (sound w.r.t.
    per-engine program order)."""
    spill_id = [0]
    for bb in nc.m.functions[0].blocks:
        new_insts = []
        for inst in bb.instructions:
            si = getattr(inst, "sync_info", None)
            if si is None or not si.on_wait:
                new_insts.append(inst)
                continue
            eng = getattr(inst, "engine", None)
            kept = list(si.on_wait)
            if len(kept) > 1 and eng is not None:
                excess, kept = kept[:-1], kept[-1:]
                for w in excess:
                    spill_id[0] += 1
                    nop = mybir.InstNoOp(
                        name=f"I-wspill-{spill_id[0]}",
                        ins=[],
                        outs=[],
                        engine=eng,
                    )
                    nop.sync_info = mybir.SyncInfo(on_wait=[w], on_update=[])
                    new_insts.append(nop)
            if len(kept) != len(si.on_wait):
                si.on_wait[:] = kept
            new_insts.append(inst)
        if len(new_insts) != len(bb.instructions):
            bb.instructions[:] = new_insts


_PHASE_MARKS = []  # (phase_label, last_inst_index_before_phase) - profiling aid


def _mark(nc, label):
    n = -1
    for fn in nc.m.functions:
        for bb in fn.blocks:
            for ins in bb.instructions:
                if ins.name.startswith("I-"):
                    try:
                        n = max(n, int(ins.name[2:]))
                    except ValueError:
                        pass
    _PHASE_MARKS.append((label, n))


B, T, D, H, HD = 2, 2048, 2048, 16, 128
NCORES = 8
HPC = H // NCORES            # heads per core = 2
M_PC = HPC * HD              # per-core feature slice = 256
BT = B * T                   # 4096
SCALE = HD ** -0.5
ROPE_THETA = 10000.0

F32 = mybir.dt.float32
F16 = mybir.dt.float16
BF16 = mybir.dt.bfloat16
BF16_NP = ml_dtypes.bfloat16

TB = 512                     # t-block for projections / q-groups
NTB_B = T // TB              # 4 t-blocks per batch
NMC = D // 128               # 16 contraction chunks
NKC = T // 128               # 16 key chunks per batch
JPG = TB // 128              # key chunks per q-group width = 4

Copy = mybir.ActivationFunctionType.Copy
Exp = mybir.ActivationFunctionType.Exp


def build_program():
    nc = bass.Bass()

    xT_d = nc.declare_dram_parameter("xT", [D, BT], BF16, isOutput=False)
    perm_d = nc.declare_dram_parameter("permM", [HD, HD], BF16, isOutput=False)
    negm_d = nc.declare_dram_parameter("negmM", [128, 128], F32, isOutput=False)
    # wq and wk concatenated so one DMA covers both (halves SP-seq time
    # on the critical startup path)
    wqk_d = nc.declare_dram_parameter(
        "wqkT", [D, 2 * M_PC], BF16, isOutput=False
    )
    wv_d = nc.declare_dram_parameter("wvT", [D, M_PC], BF16, isOutput=False)
    wo_d = nc.declare_dram_parameter("woT", [M_PC, D], BF16, isOutput=False)
    cos_d = nc.declare_dram_parameter("cosT", [HD, T], BF16, isOutput=False)
    sinh_d = nc.declare_dram_parameter("sinhT", [HD, T], BF16, isOutput=False)
    out_d = nc.declare_dram_parameter("partialT", [D, BT], F16, isOutput=True)

    xT_v = xT_d.rearrange("(c p) t -> p c t", p=128)      # [128, 16, BT]
    wqk_v = wqk_d.rearrange("(c p) n -> p c n", p=128)    # [128, 16, 512]
    wv_v = wv_d.rearrange("(c p) n -> p c n", p=128)
    wo_v = wo_d.rearrange("(c p) n -> p c n", p=128)      # [128, 2, 2048]
    out_v = out_d.rearrange("(c p) t -> p c t", p=128)    # [128, 16, BT]

    with tile.TileContext(nc) as tc:
        with (
            tc.tile_pool(name="wpool", bufs=1) as wpool,
            tc.tile_pool(name="xp", bufs=1) as xp,
            tc.tile_pool(name="big", bufs=1) as big,
            tc.tile_pool(name="rp", bufs=2) as rp,
            tc.tile_pool(name="attn_sb", bufs=6) as asb,
            tc.tile_pool(name="fs_sb", bufs=3) as fsb,
        ):
            # ---- weights + first x block, interleaved in graduated mc
            # groups (fast pipeline fill, then few big SP-cheap DMAs) ----
            wqk_sb = wpool.tile([128, NMC, 2 * M_PC], BF16, tag="wqk")
            wv_sb = wpool.tile([128, NMC, M_PC], BF16, tag="wv")
            x_tiles = {}
            xt0 = xp.tile([128, NMC, TB], BF16, tag="x0", name="x_b0_t0")
            x_tiles[(0, 0)] = xt0
            for lo, hi in ((0, 1), (1, 2), (2, 3), (3, 4), (4, 6), (6, 8),
                           (8, 10), (10, 12), (12, 14), (14, 16)):
                nc.sync.dma_start(
                    out=wqk_sb[:, lo:hi, :], in_=wqk_v[:, lo:hi, :]
                )
                # first x chunk rides the idle DVE queue, in parallel with
                # SP's weight DMA, to cut the cold-start latency
                eng = nc.scalar if lo == 0 else nc.sync
                eng.dma_start(
                    out=xt0[:, lo:hi, :], in_=xT_v[:, lo:hi, 0:TB]
                )

            cos_sb = wpool.tile([128, T], BF16, tag="cos")
            sinh_sb = wpool.tile([128, T], BF16, tag="sinh")
            perm_sb = wpool.tile([HD, HD], BF16, tag="perm")
            nc.sync.dma_start(out=perm_sb, in_=perm_d[:, :])
            nc.sync.dma_start(out=cos_sb[:, 0:TB], in_=cos_d[:, 0:TB])
            nc.sync.dma_start(out=sinh_sb[:, 0:TB], in_=sinh_d[:, 0:TB])

            def load_x(b, tb):
                t = xp.tile(
                    [128, NMC, TB], BF16, tag=f"x{tb}", name=f"x_b{b}_t{tb}"
                )
                x_tiles[(b, tb)] = t
                lo = b * T + tb * TB
                for m0 in range(0, NMC, 4):
                    nc.sync.dma_start(
                        out=t[:, m0 : m0 + 4, :],
                        in_=xT_v[:, m0 : m0 + 4, lo : lo + TB],
                    )

            # wv rides alongside tb0's V matmuls; x block 1 follows
            for m0 in range(0, NMC, 4):
                nc.sync.dma_start(
                    out=wv_sb[:, m0 : m0 + 4, :], in_=wv_v[:, m0 : m0 + 4, :]
                )
            load_x(0, 1)
            nc.sync.dma_start(out=cos_sb[:, TB:], in_=cos_d[:, TB:])
            nc.sync.dma_start(out=sinh_sb[:, TB:], in_=sinh_d[:, TB:])
            negm = wpool.tile([128, 128], F32, tag="negm")
            nc.sync.dma_start(out=negm, in_=negm_d[:, :])
            ones_col = wpool.tile([128, 1], BF16, tag="ones_c")
            nc.vector.memset(ones_col, 1.0)
            ones_row = wpool.tile([1, 128], BF16, tag="ones_r")
            nc.vector.memset(ones_row, 1.0)
            # 1/Z table: [1, HPC*T], column h*T + t (kept on partition 0)
            zrs_tab = wpool.tile([1, HPC * T], BF16, tag="zrs")

            for tb in range(2, NTB_B):
                load_x(0, tb)

            wo_sb = wpool.tile([128, HPC, D], BF16, tag="wo")
            nc.sync.dma_start(out=wo_sb, in_=wo_v)

            for b in range(B):
                t0 = b * T  # global t offset of this batch
                _mark(nc, f"b{b}_proj")

                # persistent per-batch tensors (slots reused across b)
                qT = big.tile([128, HPC, T], BF16, tag="qT")   # [hd, h, t]
                kT = big.tile([128, HPC, T], BF16, tag="kT")
                vN = big.tile([128, NKC, M_PC], BF16, tag="vN")  # [tk, j, n]
                oT = big.tile([128, HPC, T], BF16, tag="oT")   # attn out

                # ---------------- projections + RoPE ----------------
                with tc.tile_pool(name="qk_ps", bufs=1, space="PSUM") as qkp, \
                     tc.tile_pool(name="v_ps", bufs=1, space="PSUM") as vps, \
                     tc.tile_pool(name="sw_ps", bufs=2, space="PSUM") as swp:
                    for tb in range(NTB_B):
                        xt = x_tiles[(b, tb)]
                        ts_l = slice(tb * TB, (tb + 1) * TB)   # in-batch
                        ps = {}
                        for h in range(HPC):
                            for nm in ("q", "k"):
                                ps[nm, h] = qkp.tile(
                                    [128, TB], F32, tag=f"{nm}{h}",
                                    name=f"ps_{nm}{h}",
                                )
                        for mc in range(NMC):
                            for h in range(HPC):
                                for ni, nm in ((0, "q"), (1, "k")):
                                    hs = slice(
                                        ni * M_PC + h * HD,
                                        ni * M_PC + (h + 1) * HD,
                                    )
                                    nc.tensor.matmul(
                                        ps[nm, h],
                                        lhsT=wqk_sb[:, mc, hs],
                                        rhs=xt[:, mc, :],
                                        start=(mc == 0),
                                        stop=(mc == NMC - 1),
                                    )
                        # raw bf16 casts early on Act (deps ready now)
                        raws = {}
                        for nm in ("q", "k"):
                            for h in range(HPC):
                                raw = rp.tile(
                                    [128, TB], BF16, tag=f"raw{nm}{h}"
                                )
                                nc.scalar.activation(raw, ps[nm, h], Copy)
                                raws[nm, h] = raw
                        # RoPE before V: the qk/sw PSUM banks drain while the
                        # PE streams V matmuls, so the next phase's bank-WAR
                        # waits resolve before the PE gets there
                        for nm, dest in (("q", qT), ("k", kT)):
                            for h in range(HPC):
                                raw = raws[nm, h]
                                swps = swp.tile([128, TB], F32, tag="sw")
                                nc.tensor.matmul(
                                    swps, lhsT=perm_sb, rhs=raw,
                                    start=True, stop=True,
                                )
                                t2 = rp.tile([128, TB], BF16, tag="t2")
                                nc.vector.tensor_mul(t2, swps, sinh_sb[:, ts_l])
                                t1 = rp.tile([128, TB], BF16, tag="t1")
                                nc.vector.tensor_mul(t1, raw, cos_sb[:, ts_l])
                                nc.vector.tensor_add(dest[:, h, ts_l], t1, t2)
                        # V directly in natural layout: per 128-wide tk chunk
                        for s in range(JPG):
                            j = tb * JPG + s
                            pv = vps.tile(
                                [128, M_PC], F32, tag=f"v{s % 2}",
                                name=f"ps_v{s % 2}",
                            )
                            for mc in range(NMC):
                                nc.tensor.matmul(
                                    pv,
                                    lhsT=xt[:, mc, s * 128 : (s + 1) * 128],
                                    rhs=wv_sb[:, mc, :],
                                    start=(mc == 0),
                                    stop=(mc == NMC - 1),
                                )
                            nc.scalar.activation(vN[:, j, :], pv, Copy)

                if b == 0:
                    # b1's x loads: emitted here so they ride the SP queue
                    # ahead of b0's output stores
                    for tb in range(NTB_B):
                        load_x(1, tb)

                _mark(nc, f"b{b}_attn")
                # ---------------- attention ----------------
                # heads interleaved per key-chunk (both heads' score tiles
                # merged into one 2-bank PSUM tile so a single exp call
                # covers them); Z/PV trail the score/exp stream by one
                # chunk, software-pipelined ACROSS q-groups so the PE never
                # drains at a group boundary. The 1/Z broadcast borrows po
                # slots.
                with tc.tile_pool(name="st_ps", bufs=4, space="PSUM") as stp, \
                     tc.tile_pool(name="pv_ps", bufs=1, space="PSUM") as pvp, \
                     tc.tile_pool(name="z_ps", bufs=1, space="PSUM") as zpp:
                    norm_q = []  # pending (qg, h) normalization steps

                    def norm_step():
                        nqg, h = norm_q.pop(0)
                        qs0 = nqg * TB
                        zbp = pvp.tile(
                            [128, TB], F32, tag=f"po{h}", name="zbp"
                        )
                        nc.tensor.matmul(
                            zbp,
                            lhsT=ones_row,
                            rhs=zrs_tab[0:1, h * T + qs0 : h * T + qs0 + TB],
                            start=True,
                            stop=True,
                        )
                        nc.vector.tensor_mul(
                            oT[:, h, qs0 : qs0 + TB],
                            oT[:, h, qs0 : qs0 + TB],
                            zbp,
                        )
                    # (qg, j) chunk schedule, flattened
                    sched = [
                        (qg, j)
                        for qg in range(NTB_B)
                        for j in range(JPG * (qg + 1))
                    ]
                    po = {}
                    zrow = {}
                    prev = None  # (qg, j, {h: (pt, off)})

                    def zpv_step(pqg, pj, pts):
                        """Z + PV matmuls for the trailing chunk; on the last
                        chunk of a group, also emit recip + po drain."""
                        pjmax = JPG * (pqg + 1)
                        for h in range(HPC):
                            ptp, offp = pts[h]
                            nc.tensor.matmul(
                                zrow[pqg, h][:, offp:],
                                lhsT=ones_col,
                                rhs=ptp[:, offp:],
                                start=(pj == 0),
                                stop=(pj == pjmax - 1),
                            )
                            nc.tensor.matmul(
                                po[pqg, h][:, offp:],
                                lhsT=vN[:, pj, h * HD : (h + 1) * HD],
                                rhs=ptp[:, offp:],
                                start=(pj == 0),
                                stop=(pj == pjmax - 1),
                            )
                        if pj == pjmax - 1:
                            qs0 = pqg * TB
                            last_g = pqg == NTB_B - 1
                            for h in range(HPC):
                                with nc.allow_low_precision(
                                    reason="bf16 1/Z: 0.4% rel, in tolerance"
                                ):
                                    nc.vector.reciprocal(
                                        zrs_tab[
                                            0:1, h * T + qs0 : h * T + qs0 + TB
                                        ],
                                        zrow[pqg, h],
                                    )
                                if last_g:
                                    nc.scalar.activation(
                                        oT[:, h, qs0 : qs0 + TB],
                                        po[pqg, h],
                                        Copy,
                                    )
                                else:
                                    nc.vector.tensor_copy(
                                        oT[:, h, qs0 : qs0 + TB], po[pqg, h]
                                    )
                                norm_q.append((pqg, h))

                    for qg, j in sched:
                        if j == 1:
                            # previous group's 1/Z broadcasts first, so they
                            # take the po-tag slots ahead of this group's po
                            while norm_q:
                                norm_step()
                            for h in range(HPC):
                                po[qg, h] = pvp.tile(
                                    [128, TB], F32, tag=f"po{h}", name=f"po{h}"
                                )
                                zrow[qg, h] = zpp.tile(
                                    [1, TB], F32, tag=f"z{h}", name=f"z{h}"
                                )
                        qs0 = qg * TB
                        off = max(0, (j - JPG * qg) * 128)
                        ks = slice(j * 128, (j + 1) * 128)
                        cur = {}
                        for h in range(HPC):
                            st = stp.tile([128, TB], F32, tag="st")
                            nc.tensor.matmul(
                                st[:, off:],
                                lhsT=kT[:, h, ks],
                                rhs=qT[:, h, qs0 + off : qs0 + TB],
                                start=True,
                                stop=True,
                            )
                            if j >= JPG * qg:  # diagonal 128-block mask
                                nc.vector.tensor_add(
                                    st[:, off : off + 128],
                                    st[:, off : off + 128],
                                    negm,
                                )
                            pt = asb.tile([128, TB], BF16, tag="pt")
                            nc.scalar.activation(
                                pt[:, off:], st[:, off:], Exp, scale=SCALE
                            )
                            cur[h] = (pt, off)
                        if prev is not None:
                            zpv_step(*prev)
                        prev = (qg, j, cur)
                    zpv_step(*prev)
                    while norm_q:
                        norm_step()

                _mark(nc, f"b{b}_outproj")
                # ---------------- out-projection (partial) ----------------
                # output stores grouped 4 column-blocks per DMA (SP-seq time
                # per DMA is the store-path bottleneck, not bandwidth)
                with tc.tile_pool(name="fo_ps", bufs=1, space="PSUM") as fop:
                    for tb in range(NTB_B):
                        tbs = slice(tb * TB, (tb + 1) * TB)
                        fs = None
                        for nb in range(D // 128):
                            nbs = slice(nb * 128, (nb + 1) * 128)
                            fo = fop.tile(
                                [128, TB], F32, tag=f"fo{nb % 4}",
                                name=f"fo{nb % 4}",
                            )
                            for m in range(HPC):
                                nc.tensor.matmul(
                                    fo,
                                    lhsT=wo_sb[:, m, nbs],
                                    rhs=oT[:, m, tbs],
                                    start=(m == 0),
                                    stop=(m == HPC - 1),
                                )
                            grp = (
                                2
                                if (b == B - 1 and tb == NTB_B - 1 and nb >= 12)
                                else 4
                            )
                            if nb % grp == 0:
                                fs = fsb.tile(
                                    [128, 4, TB], F16, tag="fs"
                                )
                            if nb % 2 == 0:
                                nc.vector.tensor_copy(fs[:, nb % grp, :], fo)
                            else:
                                nc.scalar.activation(
                                    fs[:, nb % grp, :], fo, Copy
                                )
                            if nb % grp == grp - 1:
                                last = (
                                    b == B - 1
                                    and tb == NTB_B - 1
                                    and nb == D // 128 - 1
                                )
                                deng = nc.scalar if last else nc.sync
                                deng.dma_start(
                                    out=out_v[
                                        :,
                                        nb - grp + 1 : nb + 1,
                                        t0 + tb * TB : t0 + (tb + 1) * TB,
                                    ],
                                    in_=fs[:, 0:grp, :],
                                )
    _legalize_waits(nc)
    return nc


_NC_CACHE = None


def _get_program():
    global _NC_CACHE
    if _NC_CACHE is None:
        _NC_CACHE = build_program()
    return _NC_CACHE


def _rope_tables():
    inv_freq = 1.0 / (ROPE_THETA ** (np.arange(0, HD, 2, dtype=np.float32) / HD))
    freqs = np.arange(T, dtype=np.float32)[:, None] * inv_freq[None, :]  # (T, 64)
    emb = np.concatenate([freqs, freqs], axis=-1)                        # (T, 128)
    cosT = np.ascontiguousarray(np.cos(emb).T).astype(BF16_NP)           # [128, T]
    sinT = np.sin(emb).T.astype(np.float32)
    sinhT = np.ascontiguousarray(
        np.concatenate([-sinT[: HD // 2], sinT[HD // 2 :]], axis=0)
    ).astype(BF16_NP)
    return cosT, sinhT


def kernel(x, Wq, Wk, Wv, Wo, **run_kwargs):
    x = np.asarray(x, dtype=np.float32)
    Wq = np.asarray(Wq, dtype=np.float32)
    Wk = np.asarray(Wk, dtype=np.float32)
    Wv = np.asarray(Wv, dtype=np.float32)
    Wo = np.asarray(Wo, dtype=np.float32)

    nc = _get_program()
    cosT, sinhT = _rope_tables()
    xT = np.ascontiguousarray(x.reshape(BT, D).T).astype(BF16_NP)  # [D, BT]
    permM = np.zeros((HD, HD), dtype=BF16_NP)
    for m in range(HD):
        permM[(m + HD // 2) % HD, m] = 1.0  # out[m] = in[(m+64)%128]
    # S^T[tk, tq] causal mask for the diagonal block: keep tq(col) >= tk(row)
    r = np.arange(128)
    negmM = np.where(r[None, :] >= r[:, None], 0.0, -1e30).astype(np.float32)

    in_maps = []
    for c in range(NCORES):
        sl = slice(c * M_PC, (c + 1) * M_PC)
        in_maps.append(
            {
                "xT": xT,
                "permM": permM,
                "negmM": negmM,
                "wqkT": np.ascontiguousarray(
                    np.concatenate([Wq[sl, :].T, Wk[sl, :].T], axis=1)
                ).astype(BF16_NP),
                "wvT": np.ascontiguousarray(Wv[sl, :].T).astype(BF16_NP),
                "woT": np.ascontiguousarray(Wo[:, sl].T).astype(BF16_NP),
                "cosT": cosT,
                "sinhT": sinhT,
            }
        )

    res = run_bass_kernel_spmd(nc, in_maps, list(range(NCORES)), **run_kwargs)
    acc = np.zeros((D, BT), dtype=np.float32)
    for c in range(NCORES):
        acc += res.results[c]["partialT"].astype(np.float32)
    out = np.ascontiguousarray(acc.T).reshape(B, T, D)
    if run_kwargs:
        return out, res
    return out


# revision 38
# speedup vs baseline: 1.4030x; 1.0200x over previous
"""Multi-head self-attention (B=2, T=2048, D=2048, H=16, RoPE, causal)
as a Bass/Tile kernel running SPMD on 8 trn2 NeuronCores.

Sharding: tensor-parallel over heads (2 heads per core). Each core
computes its heads' Q/K/V projections, RoPE, causal attention, and a
partial out-projection over its 256 feature columns; the host sums the
8 partial outputs (all-reduce equivalent).

Dataflow (per core, per batch):
  - x streamed per 512-wide t-block ([128, 16, 512] SBUF tiles, 4 tags);
    the first block's DMA is interleaved per-contraction-chunk with the
    weight loads so the PE starts ~2us in.
  - Q/K projections in "T-layout" (feature dim on partitions, time on
    free); RoPE rotate-half via a PE permutation matmul, combines on DVE
    in bf16 (2x mode where operands allow).
  - V projected directly in natural layout ([tk, d]): lhsT = x chunk,
    rhs = Wv slice -- no PE transposes.
  - scores computed transposed: S^T[tk, tq] per (key-chunk, q-group).
    Chunks are narrowed to the causal region (exact 136-block lower
    triangle, no fully-masked work); only the diagonal 128x128 block
    gets a mask add. The two heads' chunk streams are interleaved so
    the PE always has ~1.3us of work while exp round-trips through
    DVE/Act. Z row sums via a [128,1] ones matmul accumulated in PSUM.
  - normalization trails each q-group: po -> oT (unnormalized cast),
    1/Z table via DVE reciprocal, then a ones-row broadcast matmul
    (riding the po PSUM slots between groups) + in-place DVE multiply.
  - out-projection accumulates the two head-chunks in PSUM; partial
    result cast to f16 and DMA'd out; host sums partials across cores.
"""

import sys

sys.path.insert(0, "/opt/trn_rl_repo")

import ml_dtypes
import numpy as np

import concourse.bass as bass
import concourse.mybir as mybir
import concourse.tile as tile
from concourse.bass_utils import run_bass_kernel_spmd


def _legalize_waits(nc):
    """Walrus codegen rejects >2 sync waits on DMA/matmul/nop-class
    instructions, and Tile's pool-recycle waits bypass its own elision.
    Spill excess waits (>1) onto freshly inserted same-engine NoOps
    placed immediately before the offending instruction (sound w.r.t.
    per-engine program order)."""
    spill_id = [0]
    for bb in nc.m.functions[0].blocks:
        new_insts = []
        for inst in bb.instructions:
            si = getattr(inst, "sync_info", None)
            if si is None or not si.on_wait:
                new_insts.append(inst)
                continue
            eng = getattr(inst, "engine", None)
            kept = list(si.on_wait)
            if len(kept) > 1 and eng is not None:
                excess, kept = kept[:-1], kept[-1:]
                for w in excess:
                    spill_id[0] += 1
                    nop = mybir.InstNoOp(
                        name=f"I-wspill-{spill_id[0]}",
                        ins=[],
                        outs=[],
                        engine=eng,
                    )
                    nop.sync_info = mybir.SyncInfo(on_wait=[w], on_update=[])
                    new_insts.append(nop)
            if len(kept) != len(si.on_wait):
                si.on_wait[:] = kept
            new_insts.append(inst)
        if len(new_insts) != len(bb.instructions):
            bb.instructions[:] = new_insts


_PHASE_MARKS = []  # (phase_label, last_inst_index_before_phase) - profiling aid


def _mark(nc, label):
    n = -1
    for fn in nc.m.functions:
        for bb in fn.blocks:
            for ins in bb.instructions:
                if ins.name.startswith("I-"):
                    try:
                        n = max(n, int(ins.name[2:]))
                    except ValueError:
                        pass
    _PHASE_MARKS.append((label, n))


B, T, D, H, HD = 2, 2048, 2048, 16, 128
NCORES = 8
HPC = H // NCORES            # heads per core = 2
M_PC = HPC * HD              # per-core feature slice = 256
BT = B * T                   # 4096
SCALE = HD ** -0.5
ROPE_THETA = 10000.0

F32 = mybir.dt.float32
F16 = mybir.dt.float16
BF16 = mybir.dt.bfloat16
BF16_NP = ml_dtypes.bfloat16

TB = 512                     # t-block for projections / q-groups
NTB_B = T // TB              # 4 t-blocks per batch
NMC = D // 128               # 16 contraction chunks
NKC = T // 128               # 16 key chunks per batch
JPG = TB // 128              # key chunks per q-group width = 4

Copy = mybir.ActivationFunctionType.Copy
Exp = mybir.ActivationFunctionType.Exp


def build_program():
    nc = bass.Bass()

    xT_d = nc.declare_dram_parameter("xT", [D, BT], BF16, isOutput=False)
    perm_d = nc.declare_dram_parameter("permM", [HD, HD], BF16, isOutput=False)
    negm_d = nc.declare_dram_parameter("negmM", [128, 128], F32, isOutput=False)
    # wq and wk concatenated so one DMA covers both (halves SP-seq time
    # on the critical startup path)
    wqk_d = nc.declare_dram_parameter(
        "wqkT", [D, 2 * M_PC], BF16, isOutput=False
    )
    wv_d = nc.declare_dram_parameter("wvT", [D, M_PC], BF16, isOutput=False)
    wo_d = nc.declare_dram_parameter("woT", [M_PC, D], BF16, isOutput=False)
    cos_d = nc.declare_dram_parameter("cosT", [HD, T], BF16, isOutput=False)
    sinh_d = nc.declare_dram_parameter("sinhT", [HD, T], BF16, isOutput=False)
    out_d = nc.declare_dram_parameter("partialT", [D, BT], F16, isOutput=True)

    xT_v = xT_d.rearrange("(c p) t -> p c t", p=128)      # [128, 16, BT]
    wqk_v = wqk_d.rearrange("(c p) n -> p c n", p=128)    # [128, 16, 512]
    wv_v = wv_d.rearrange("(c p) n -> p c n", p=128)
    wo_v = wo_d.rearrange("(c p) n -> p c n", p=128)      # [128, 2, 2048]
    out_v = out_d.rearrange("(c p) t -> p c t", p=128)    # [128, 16, BT]

    with tile.TileContext(nc) as tc:
        with (
            tc.tile_pool(name="wpool", bufs=1) as wpool,
            tc.tile_pool(name="xp", bufs=1) as xp,
            tc.tile_pool(name="big", bufs=1) as big,
            tc.tile_pool(name="rp", bufs=2) as rp,
            tc.tile_pool(name="attn_sb", bufs=6) as asb,
            tc.tile_pool(name="fs_sb", bufs=3) as fsb,
        ):
            # ---- weights + first x block, interleaved in graduated mc
            # groups (fast pipeline fill, then few big SP-cheap DMAs) ----
            wqk_sb = wpool.tile([128, NMC, 2 * M_PC], BF16, tag="wqk")
            wv_sb = wpool.tile([128, NMC, M_PC], BF16, tag="wv")
            x_tiles = {}
            xt0 = xp.tile([128, NMC, TB], BF16, tag="x0", name="x_b0_t0")
            x_tiles[(0, 0)] = xt0
            for lo, hi in ((0, 1), (1, 2), (2, 3), (3, 4), (4, 6), (6, 8),
                           (8, 10), (10, 12), (12, 14), (14, 16)):
                nc.sync.dma_start(
                    out=wqk_sb[:, lo:hi, :], in_=wqk_v[:, lo:hi, :]
                )
                # first x chunk rides the idle DVE queue, in parallel with
                # SP's weight DMA, to cut the cold-start latency
                eng = nc.scalar if lo == 0 else nc.sync
                eng.dma_start(
                    out=xt0[:, lo:hi, :], in_=xT_v[:, lo:hi, 0:TB]
                )

            cos_sb = wpool.tile([128, T], BF16, tag="cos")
            sinh_sb = wpool.tile([128, T], BF16, tag="sinh")
            perm_sb = wpool.tile([HD, HD], BF16, tag="perm")
            nc.sync.dma_start(out=perm_sb, in_=perm_d[:, :])
            nc.sync.dma_start(out=cos_sb[:, 0:TB], in_=cos_d[:, 0:TB])
            nc.sync.dma_start(out=sinh_sb[:, 0:TB], in_=sinh_d[:, 0:TB])

            def load_x(b, tb):
                t = xp.tile(
                    [128, NMC, TB], BF16, tag=f"x{tb}", name=f"x_b{b}_t{tb}"
                )
                x_tiles[(b, tb)] = t
                lo = b * T + tb * TB
                for m0 in range(0, NMC, 4):
                    nc.sync.dma_start(
                        out=t[:, m0 : m0 + 4, :],
                        in_=xT_v[:, m0 : m0 + 4, lo : lo + TB],
                    )

            # wv rides alongside tb0's V matmuls; x block 1 follows
            for m0 in range(0, NMC, 4):
                nc.sync.dma_start(
                    out=wv_sb[:, m0 : m0 + 4, :], in_=wv_v[:, m0 : m0 + 4, :]
                )
            load_x(0, 1)
            nc.sync.dma_start(out=cos_sb[:, TB:], in_=cos_d[:, TB:])
            nc.sync.dma_start(out=sinh_sb[:, TB:], in_=sinh_d[:, TB:])
            negm = wpool.tile([128, 128], F32, tag="negm")
            nc.sync.dma_start(out=negm, in_=negm_d[:, :])
            ones_col = wpool.tile([128, 1], BF16, tag="ones_c")
            nc.vector.memset(ones_col, 1.0)
            ones_row = wpool.tile([1, 128], BF16, tag="ones_r")
            nc.vector.memset(ones_row, 1.0)
            # 1/Z table: [1, HPC*T], column h*T + t (kept on partition 0)
            zrs_tab = wpool.tile([1, HPC * T], BF16, tag="zrs")

            for tb in range(2, NTB_B):
                load_x(0, tb)

            wo_sb = wpool.tile([128, HPC, D], BF16, tag="wo")
            nc.sync.dma_start(out=wo_sb, in_=wo_v)

            for b in range(B):
                t0 = b * T  # global t offset of this batch
                _mark(nc, f"b{b}_proj")

                # persistent per-batch tensors (slots reused across b)
                qT = big.tile([128, HPC, T], BF16, tag="qT")   # [hd, h, t]
                kT = big.tile([128, HPC, T], BF16, tag="kT")
                vN = big.tile([128, NKC, M_PC], BF16, tag="vN")  # [tk, j, n]
                oT = big.tile([128, HPC, T], BF16, tag="oT")   # attn out

                # ---------------- projections + RoPE ----------------
                with tc.tile_pool(name="qk_ps", bufs=1, space="PSUM") as qkp, \
                     tc.tile_pool(name="v_ps", bufs=1, space="PSUM") as vps:
                    for tb in range(NTB_B):
                        xt = x_tiles[(b, tb)]
                        ts_l = slice(tb * TB, (tb + 1) * TB)   # in-batch
                        ps = {}
                        for h in range(HPC):
                            for nm in ("q", "k"):
                                ps[nm, h] = qkp.tile(
                                    [128, TB], F32, tag=f"{nm}{h}",
                                    name=f"ps_{nm}{h}",
                                )
                        for mc in range(NMC):
                            for h in range(HPC):
                                for ni, nm in ((0, "q"), (1, "k")):
                                    hs = slice(
                                        ni * M_PC + h * HD,
                                        ni * M_PC + (h + 1) * HD,
                                    )
                                    nc.tensor.matmul(
                                        ps[nm, h],
                                        lhsT=wqk_sb[:, mc, hs],
                                        rhs=xt[:, mc, :],
                                        start=(mc == 0),
                                        stop=(mc == NMC - 1),
                                    )
                        # raw bf16 casts early on Act (deps ready now)
                        raws = {}
                        for nm in ("q", "k"):
                            for h in range(HPC):
                                raw = rp.tile(
                                    [128, TB], BF16, tag=f"raw{nm}{h}"
                                )
                                nc.scalar.activation(raw, ps[nm, h], Copy)
                                raws[nm, h] = raw
                        # RoPE before V: the qk/sw PSUM banks drain while the
                        # PE streams V matmuls, so the next phase's bank-WAR
                        # waits resolve before the PE gets there
                        for nm, dest in (("q", qT), ("k", kT)):
                            for h in range(HPC):
                                raw = raws[nm, h]
                                swb = rp.tile([128, TB], BF16, tag="swb")
                                nc.sync.dma_start(
                                    out=swb[0:64, :], in_=raw[64:128, :]
                                )
                                nc.sync.dma_start(
                                    out=swb[64:128, :], in_=raw[0:64, :]
                                )
                                t2 = rp.tile([128, TB], BF16, tag="t2")
                                nc.vector.tensor_mul(t2, swb, sinh_sb[:, ts_l])
                                t1 = rp.tile([128, TB], BF16, tag="t1")
                                nc.vector.tensor_mul(t1, raw, cos_sb[:, ts_l])
                                nc.vector.tensor_add(dest[:, h, ts_l], t1, t2)
                        # V directly in natural layout: per 128-wide tk chunk
                        for s in range(JPG):
                            j = tb * JPG + s
                            pv = vps.tile(
                                [128, M_PC], F32, tag=f"v{s % 2}",
                                name=f"ps_v{s % 2}",
                            )
                            for mc in range(NMC):
                                nc.tensor.matmul(
                                    pv,
                                    lhsT=xt[:, mc, s * 128 : (s + 1) * 128],
                                    rhs=wv_sb[:, mc, :],
                                    start=(mc == 0),
                                    stop=(mc == NMC - 1),
                                )
                            nc.scalar.activation(vN[:, j, :], pv, Copy)

                if b == 0:
                    # b1's x loads: emitted here so they ride the SP queue
                    # ahead of b0's output stores
                    for tb in range(NTB_B):
                        load_x(1, tb)

                _mark(nc, f"b{b}_attn")
                # ---------------- attention ----------------
                # heads interleaved per key-chunk (both heads' score tiles
                # merged into one 2-bank PSUM tile so a single exp call
                # covers them); Z/PV trail the score/exp stream by one
                # chunk, software-pipelined ACROSS q-groups so the PE never
                # drains at a group boundary. The 1/Z broadcast borrows po
                # slots.
                op_state = {"tb": 0, "nb": 0, "fs": None, "done": {}}

                def op_step(fop, fsq, grp, deng_last=False):
                    """One out-projection column block: 2 accumulating
                    matmuls + f16 cast (DVE) + grouped store."""
                    tb, nb = op_state["tb"], op_state["nb"]
                    if tb >= NTB_B:
                        return False
                    tbs = slice(tb * TB, (tb + 1) * TB)
                    nbs = slice(nb * 128, (nb + 1) * 128)
                    fo = fop.tile([128, TB], F32, tag="fo", name="fo")
                    for m in range(HPC):
                        nc.tensor.matmul(
                            fo,
                            lhsT=wo_sb[:, m, nbs],
                            rhs=oT[:, m, tbs],
                            start=(m == 0),
                            stop=(m == HPC - 1),
                        )
                    if nb % grp == 0:
                        op_state["fs"] = fsq.tile([128, 4, TB], F16, tag="fs", name="fs")
                    fs = op_state["fs"]
                    if nb % 2 == 0:
                        nc.vector.tensor_copy(fs[:, nb % grp, :], fo)
                    else:
                        nc.scalar.activation(fs[:, nb % grp, :], fo, Copy)
                    if nb % grp == grp - 1:
                        deng = nc.scalar if deng_last else nc.sync
                        deng.dma_start(
                            out=out_v[
                                :,
                                nb - grp + 1 : nb + 1,
                                t0 + tb * TB : t0 + (tb + 1) * TB,
                            ],
                            in_=fs[:, 0:grp, :],
                        )
                    nb += 1
                    if nb == D // 128:
                        nb = 0
                        tb += 1
                    op_state["tb"], op_state["nb"] = tb, nb
                    return True

                with tc.tile_pool(name="st_ps", bufs=3, space="PSUM") as stp, \
                     tc.tile_pool(name="pv_ps", bufs=1, space="PSUM") as pvp, \
                     tc.tile_pool(name="z_ps", bufs=1, space="PSUM") as zpp, \
                     tc.tile_pool(name="fo_i", bufs=1, space="PSUM") as foi:
                    norm_q = []  # pending (qg, h) normalization steps

                    def norm_step():
                        nqg, h = norm_q.pop(0)
                        qs0 = nqg * TB
                        zbp = pvp.tile(
                            [128, TB], F32, tag=f"po{h}", name="zbp"
                        )
                        nc.tensor.matmul(
                            zbp,
                            lhsT=ones_row,
                            rhs=zrs_tab[0:1, h * T + qs0 : h * T + qs0 + TB],
                            start=True,
                            stop=True,
                        )
                        nc.vector.tensor_mul(
                            oT[:, h, qs0 : qs0 + TB],
                            oT[:, h, qs0 : qs0 + TB],
                            zbp,
                        )
                    # (qg, j) chunk schedule, flattened
                    sched = [
                        (qg, j)
                        for qg in range(NTB_B)
                        for j in range(JPG * (qg + 1))
                    ]
                    po = {}
                    zrow = {}
                    prev = None  # (qg, j, {h: (pt, off)})

                    def zpv_step(pqg, pj, pts):
                        """Z + PV matmuls for the trailing chunk; on the last
                        chunk of a group, also emit recip + po drain."""
                        pjmax = JPG * (pqg + 1)
                        for h in range(HPC):
                            ptp, offp = pts[h]
                            nc.tensor.matmul(
                                zrow[pqg, h][:, offp:],
                                lhsT=ones_col,
                                rhs=ptp[:, offp:],
                                start=(pj == 0),
                                stop=(pj == pjmax - 1),
                            )
                            nc.tensor.matmul(
                                po[pqg, h][:, offp:],
                                lhsT=vN[:, pj, h * HD : (h + 1) * HD],
                                rhs=ptp[:, offp:],
                                start=(pj == 0),
                                stop=(pj == pjmax - 1),
                            )
                        if pj == pjmax - 1:
                            qs0 = pqg * TB
                            last_g = pqg == NTB_B - 1
                            for h in range(HPC):
                                with nc.allow_low_precision(
                                    reason="bf16 1/Z: 0.4% rel, in tolerance"
                                ):
                                    nc.vector.reciprocal(
                                        zrs_tab[
                                            0:1, h * T + qs0 : h * T + qs0 + TB
                                        ],
                                        zrow[pqg, h],
                                    )
                                if last_g:
                                    nc.scalar.activation(
                                        oT[:, h, qs0 : qs0 + TB],
                                        po[pqg, h],
                                        Copy,
                                    )
                                else:
                                    nc.vector.tensor_copy(
                                        oT[:, h, qs0 : qs0 + TB], po[pqg, h]
                                    )
                                norm_q.append((pqg, h))

                    for qg, j in sched:
                        if j == 1:
                            # previous group's 1/Z broadcasts first, so they
                            # take the po-tag slots ahead of this group's po
                            while norm_q:
                                norm_step()
                            for h in range(HPC):
                                po[qg, h] = pvp.tile(
                                    [128, TB], F32, tag=f"po{h}", name=f"po{h}"
                                )
                                zrow[qg, h] = zpp.tile(
                                    [1, TB], F32, tag=f"z{h}", name=f"z{h}"
                                )
                        qs0 = qg * TB
                        off = max(0, (j - JPG * qg) * 128)
                        ks = slice(j * 128, (j + 1) * 128)
                        cur = {}
                        for h in range(HPC):
                            st = stp.tile([128, TB], F32, tag="st")
                            nc.tensor.matmul(
                                st[:, off:],
                                lhsT=kT[:, h, ks],
                                rhs=qT[:, h, qs0 + off : qs0 + TB],
                                start=True,
                                stop=True,
                            )
                            if j >= JPG * qg:  # diagonal 128-block mask
                                nc.vector.tensor_add(
                                    st[:, off : off + 128],
                                    st[:, off : off + 128],
                                    negm,
                                )
                            pt = asb.tile([128, TB], BF16, tag="pt")
                            nc.scalar.activation(
                                pt[:, off:], st[:, off:], Exp, scale=SCALE
                            )
                            cur[h] = (pt, off)
                        if prev is not None:
                            zpv_step(*prev)
                        if qg >= 1 and j >= 2 and op_state["tb"] < qg:
                            op_step(foi, fsb, 4)
                        prev = (qg, j, cur)
                    zpv_step(*prev)
                    while norm_q:
                        norm_step()
                    while norm_q:
                        norm_step()

                _mark(nc, f"b{b}_outproj")
                # ---------------- out-projection drain ----------------
                with tc.tile_pool(name="fo_ps", bufs=1, space="PSUM") as fop:
                    n_left = (NTB_B - op_state["tb"]) * (D // 128) - op_state["nb"]
                    k = 0
                    while True:
                        k += 1
                        lastish = b == B - 1 and k > n_left - 4
                        if not op_step(
                            fop, fsb, 2 if lastish else 4,
                            deng_last=(b == B - 1 and k == n_left),
                        ):
                            break
    _legalize_waits(nc)
    return nc


_NC_CACHE = None


def _get_program():
    global _NC_CACHE
    if _NC_CACHE is None:
        _NC_CACHE = build_program()
    return _NC_CACHE


def _rope_tables():
    inv_freq = 1.0 / (ROPE_THETA ** (np.arange(0, HD, 2, dtype=np.float32) / HD))
    freqs = np.arange(T, dtype=np.float32)[:, None] * inv_freq[None, :]  # (T, 64)
    emb = np.concatenate([freqs, freqs], axis=-1)                        # (T, 128)
    cosT = np.ascontiguousarray(np.cos(emb).T).astype(BF16_NP)           # [128, T]
    sinT = np.sin(emb).T.astype(np.float32)
    sinhT = np.ascontiguousarray(
        np.concatenate([-sinT[: HD // 2], sinT[HD // 2 :]], axis=0)
    ).astype(BF16_NP)
    return cosT, sinhT


def kernel(x, Wq, Wk, Wv, Wo, **run_kwargs):
    x = np.asarray(x, dtype=np.float32)
    Wq = np.asarray(Wq, dtype=np.float32)
    Wk = np.asarray(Wk, dtype=np.float32)
    Wv = np.asarray(Wv, dtype=np.float32)
    Wo = np.asarray(Wo, dtype=np.float32)

    nc = _get_program()
    cosT, sinhT = _rope_tables()
    xT = np.ascontiguousarray(x.reshape(BT, D).T).astype(BF16_NP)  # [D, BT]
    permM = np.zeros((HD, HD), dtype=BF16_NP)
    for m in range(HD):
        permM[(m + HD // 2) % HD, m] = 1.0  # out[m] = in[(m+64)%128]
    # S^T[tk, tq] causal mask for the diagonal block: keep tq(col) >= tk(row)
    r = np.arange(128)
    negmM = np.where(r[None, :] >= r[:, None], 0.0, -1e30).astype(np.float32)

    in_maps = []
    for c in range(NCORES):
        sl = slice(c * M_PC, (c + 1) * M_PC)
        in_maps.append(
            {
                "xT": xT,
                "permM": permM,
                "negmM": negmM,
                "wqkT": np.ascontiguousarray(
                    np.concatenate([Wq[sl, :].T, Wk[sl, :].T], axis=1)
                ).astype(BF16_NP),
                "wvT": np.ascontiguousarray(Wv[sl, :].T).astype(BF16_NP),
                "woT": np.ascontiguousarray(Wo[:, sl].T).astype(BF16_NP),
                "cosT": cosT,
                "sinhT": sinhT,
            }
        )

    res = run_bass_kernel_spmd(nc, in_maps, list(range(NCORES)), **run_kwargs)
    acc = np.zeros((D, BT), dtype=np.float32)
    for c in range(NCORES):
        acc += res.results[c]["partialT"].astype(np.float32)
    out = np.ascontiguousarray(acc.T).reshape(B, T, D)
    if run_kwargs:
        return out, res
    return out
